# revision 1
# baseline (speedup 1.0000x reference)
"""Distributed Trainium2 kernel for the gated-adapter attention module.

Head-parallel tensor parallelism over 8 NeuronCores (4 heads each):
wq/wk/wv sharded by output channels; attention computed per head in
S^T orientation (keys on partitions) so every matmul streams 512-wide;
an AllToAll converts the attention output from head-sharded to
token-sharded so each core applies the full wo projection to its
512-token chunk. All operand transposes go through the DMA xbar
(bf16) instead of the PE array. Compute bf16, f32 PSUM accumulation.
DRAM intermediates are split per-panel so the Tile scheduler can
overlap staging, compute, and spills across phases.
"""

import sys

sys.path.insert(0, "/opt/trn_rl_repo")

import numpy as np

import concourse.bass as bass
import concourse.mybir as mybir
import concourse.tile as tile
from concourse import bacc, bass_utils
from concourse.bass import ds, ts
from concourse.masks import make_identity

N_CORES = 8
B, S, D = 2, 2048, 4096
H = 32
HD = 128                      # head dim
H_LOC = H // N_CORES          # 4 heads per core
CH = H_LOC * HD               # 512 local channels
TOK = B * S                   # 4096 tokens
NK = D // 128                 # 32 contraction tiles
AL = 10                       # adapter length
TPC = TOK // N_CORES          # 512 tokens per core after AllToAll
NQC = S // 512                # 4 query chunks per sequence
NPAN = TOK // 512             # 8 token panels
SCALE = 1.0 / float(np.sqrt(HD))
BF = mybir.dt.bfloat16
F32 = mybir.dt.float32
EXP = mybir.ActivationFunctionType.Exp
TANH = mybir.ActivationFunctionType.Tanh
MULT = mybir.AluOpType.mult
ADD = mybir.AluOpType.add


def build():
    nc = bacc.Bacc("TRN2", target_bir_lowering=False, debug=False,
                   num_devices=N_CORES)
    x = nc.dram_tensor("x", [TOK, D], F32, kind="ExternalInput")
    wq = nc.dram_tensor("wq", [CH, D], F32, kind="ExternalInput")
    wk = nc.dram_tensor("wk", [CH, D], F32, kind="ExternalInput")
    wv = nc.dram_tensor("wv", [CH, D], F32, kind="ExternalInput")
    wo = nc.dram_tensor("wo", [D, D], F32, kind="ExternalInput")
    gate = nc.dram_tensor("gate", [1, H_LOC], F32, kind="ExternalInput")
    adapter = nc.dram_tensor("adapter", [AL, D], F32, kind="ExternalInput")
    fcos = nc.dram_tensor("fcos", [S, HD // 2], F32, kind="ExternalInput")
    fsin = nc.dram_tensor("fsin", [S, HD // 2], F32, kind="ExternalInput")
    mask = nc.dram_tensor("mask", [S, S], F32, kind="ExternalInput")
    out = nc.dram_tensor("out", [TPC, D], F32, kind="ExternalOutput")

    with tile.TileContext(nc) as tc:
        with tc.tile_pool(name="dram", bufs=1, space="DRAM") as dram, \
             tc.tile_pool(name="persist", bufs=1) as persist:
            wb_ds = [dram.tile([CH, D], BF, tag=f"wb{i}", name=f"wb{i}")
                     for i in range(3)]
            wob_d = dram.tile([D, D], BF, tag="wob_d")
            woT_d = dram.tile([D, D], BF, tag="woT_d")
            qn_ds = [dram.tile([S, CH], BF, tag=f"qn{b}", name=f"qn{b}")
                     for b in range(B)]
            kn_ds = [dram.tile([S, CH], BF, tag=f"kn{b}", name=f"kn{b}")
                     for b in range(B)]
            v_ds = [dram.tile([S, CH], BF, tag=f"vn{b}", name=f"vn{b}")
                    for b in range(B)]
            oT_d = dram.tile([CH, TOK], BF, tag="oT_d")
            a2a_in = dram.tile([N_CORES, CH, TPC], BF, tag="a2a_in")
            a2a_out = dram.tile([N_CORES, CH, TPC], BF, tag="a2a_out")

            ident = persist.tile([128, 128], BF, tag="ident")
            make_identity(nc, ident[:])
            ones = persist.tile([128, 128], BF, tag="ones")
            nc.vector.memset(ones[:], 1.0)
            g_sb = persist.tile([128, H_LOC], F32, tag="g_sb")
            g_in = persist.tile([128, H_LOC], F32, tag="g_in")
            nc.scalar.dma_start(g_in[:], gate.ap().partition_broadcast(128))
            nc.scalar.activation(g_sb[:], g_in[:], TANH)
            a_kT = persist.tile([128, H_LOC, AL], BF, tag="a_kT")
            a_v = persist.tile([AL, H_LOC, HD], BF, tag="a_v")
            maskT = persist.tile([128, S // 128, 128], BF, tag="maskT")
            cs_all = persist.tile([128, S // 128, HD // 2], F32, tag="cs_all")
            nc.scalar.dma_start(
                cs_all[:], fcos.ap().rearrange("(pb p) f -> p pb f", p=128))
            sn_all = persist.tile([128, S // 128, HD // 2], F32, tag="sn_all")
            nc.scalar.dma_start(
                sn_all[:], fsin.ap().rearrange("(pb p) f -> p pb f", p=128))

            # ================= phase 1: QKV (single pass) =================
            with tc.tile_pool(name="wph", bufs=1) as wph, \
                 tc.tile_pool(name="pst", bufs=2, space="PSUM") as pst, \
                 tc.tile_pool(name="psb", bufs=2, space="PSUM") as psb:
                aT = persist.tile([128, NK, AL], BF, tag="aT")
                with tc.tile_pool(name="stg", bufs=2) as stg:
                    # mask^T diagonal blocks (PE transpose, bf16)
                    for dblk in range(S // 128):
                        mdf = stg.tile([128, 128], F32, tag="mdf")
                        nc.scalar.dma_start(
                            mdf[:], mask.ap()[ts(dblk, 128), ts(dblk, 128)])
                        mdb = stg.tile([128, 128], BF, tag="mdb")
                        nc.vector.tensor_copy(mdb[:], mdf[:])
                        mps = pst.tile([128, 128], BF, tag="mps")
                        nc.tensor.transpose(mps[:], mdb[:], ident[:])
                        nc.vector.tensor_copy(maskT[:, dblk, :], mps[:])
                    # adapter^T [128 dim, AL] tiles (PE transpose, bf16)
                    ab = stg.tile([AL, D], BF, tag="ab", bufs=1)
                    for hf in range(4):
                        af = stg.tile([AL, D // 4], F32, tag="af")
                        nc.scalar.dma_start(af[:],
                                            adapter.ap()[:, ts(hf, D // 4)])
                        nc.vector.tensor_copy(ab[:, ts(hf, D // 4)], af[:])
                    for dt in range(NK):
                        aps = pst.tile([128, 128], BF, tag="mps")
                        nc.tensor.transpose(aps[:, :AL], ab[:, ts(dt, 128)],
                                            ident[:AL, :AL])
                        nc.vector.tensor_copy(aT[:, dt, :], aps[:, :AL])
                    # stage bf16 copies of wq/wk/wv in DRAM
                    for p_i, wt in ((0, wq), (1, wk), (2, wv)):
                        for cs in range(CH // 128):
                            for hf in range(2):
                                wf = stg.tile([128, D // 2], F32, tag="wf",
                                              bufs=2)
                                wbt = stg.tile([128, D // 2], BF, tag="wbt",
                                               bufs=2)
                                nc.scalar.dma_start(
                                    wf[:], wt.ap()[ts(cs, 128), ts(hf, D // 2)])
                                nc.vector.tensor_copy(wbt[:], wf[:])
                                nc.sync.dma_start(
                                    wb_ds[p_i][ts(cs, 128), ts(hf, D // 2)],
                                    wbt[:])
                    # stage x as bf16, one DRAM tile per 512-token panel
                # load all three transposed weight sets [128, NK, CH]
                wTs = []
                for p_i in range(3):
                    wT = wph.tile([128, NK, CH], BF, tag=f"wT{p_i}",
                                  name=f"wT{p_i}")
                    wTs.append(wT)
                    for dt in range(NK):
                        nc.sync.dma_start_transpose(
                            wT[:, dt, :], wb_ds[p_i][:, ts(dt, 128)])
                # a_k^T [ch, AL] per head, a_v [AL, ch]
                for cs in range(H_LOC):
                    pk = psb.tile([128, CH], F32, tag="ppq")
                    for dt in range(NK):
                        nc.tensor.matmul(pk[:, :AL],
                                         lhsT=wTs[1][:, dt, ts(cs, 128)],
                                         rhs=aT[:, dt, :], start=(dt == 0),
                                         stop=(dt == NK - 1))
                    nc.vector.tensor_copy(a_kT[:, cs, :], pk[:, :AL])
                pv = psb.tile([128, CH], F32, tag="ppq")
                for dt in range(NK):
                    nc.tensor.matmul(pv[:AL, :], lhsT=aT[:, dt, :],
                                     rhs=wTs[2][:, dt, :], start=(dt == 0),
                                     stop=(dt == NK - 1))
                for cs in range(H_LOC):
                    nc.vector.tensor_copy(a_v[:, cs, :], pv[:AL, ts(cs, 128)])

                # main QKV: quarter-panels of 512 tokens
                with tc.tile_pool(name="run", bufs=2) as st:
                    for qp in range(NPAN):
                        b_i, prow = qp // NQC, (qp % NQC) * 512
                        # load + cast this panel, transpose on the PE
                        xT = st.tile([128, NK, 512], BF, tag="xT")
                        for sp_i in range(4):
                            tstr = qp * 4 + sp_i
                            for hf in range(4):
                                xf = st.tile([128, D // 4], F32, tag="xf",
                                             bufs=2)
                                xbt = st.tile([128, D // 4], BF, tag="xbt",
                                              bufs=2)
                                nc.scalar.dma_start(
                                    xf[:],
                                    x.ap()[ts(tstr, 128), ts(hf, D // 4)])
                                nc.vector.tensor_copy(xbt[:], xf[:])
                                for dtl in range(NK // 4):
                                    dt = hf * (NK // 4) + dtl
                                    xtp = pst.tile([128, 128], BF, tag="mps")
                                    nc.tensor.transpose(
                                        xtp[:], xbt[:, ts(dtl, 128)], ident[:])
                                    nc.vector.tensor_copy(
                                        xT[:, dt, ts(sp_i, 128)], xtp[:])
                        # wo staging rides along, one eighth per panel
                        for dstr in range(qp * 4, qp * 4 + 4):
                            for hf in range(4):
                                wof = st.tile([128, D // 4], F32, tag="wof",
                                              bufs=2)
                                wob = st.tile([128, D // 4], BF, tag="wob",
                                              bufs=2)
                                nc.scalar.dma_start(
                                    wof[:],
                                    wo.ap()[ts(dstr, 128), ts(hf, D // 4)])
                                nc.vector.tensor_copy(wob[:], wof[:])
                                nc.sync.dma_start(
                                    wob_d[ts(dstr, 128), ts(hf, D // 4)],
                                    wob[:])
                        for sp_i in range(4):
                            srow = prow + sp_i * 128
                            pps = [psb.tile([128, CH], F32, tag=f"pp{pn}",
                                            name=f"pp{pn}") for pn in "qkv"]
                            for dt in range(NK):
                                for p_i in range(3):
                                    nc.tensor.matmul(
                                        pps[p_i][:],
                                        lhsT=xT[:, dt, ts(sp_i, 128)],
                                        rhs=wTs[p_i][:, dt, :],
                                        start=(dt == 0), stop=(dt == NK - 1))
                            # v: plain cast+store
                            vb = st.tile([128, CH], BF, tag="vb")
                            nc.vector.tensor_copy(vb[:], pps[2][:])
                            nc.scalar.dma_start(v_ds[b_i][ds(srow, 128), :],
                                                vb[:])
                            # q, k: RoPE then store natural
                            csb = cs_all[:, (srow // 128) % (S // 128), :]
                            ssb = sn_all[:, (srow // 128) % (S // 128), :]
                            for p_i, dstl in ((0, qn_ds), (1, kn_ds)):
                                rp = st.tile([128, CH], BF, tag=f"rp{p_i}",
                                             name=f"rp{p_i}")
                                for h in range(H_LOC):
                                    pv2 = pps[p_i][:, ts(h, HD)].rearrange(
                                        "p (i two) -> p two i", two=2)
                                    rv = rp[:, ts(h, HD)].rearrange(
                                        "p (i two) -> p two i", two=2)
                                    a0, b0 = pv2[:, 0, :], pv2[:, 1, :]
                                    t1 = st.tile([128, HD // 2], F32, tag="t1")
                                    t2 = st.tile([128, HD // 2], F32, tag="t2")
                                    nc.vector.tensor_mul(t1[:], a0, csb)
                                    nc.vector.tensor_mul(t2[:], b0, ssb)
                                    nc.vector.tensor_sub(rv[:, 0, :],
                                                         t1[:], t2[:])
                                    nc.vector.tensor_mul(t1[:], a0, ssb)
                                    nc.vector.tensor_mul(t2[:], b0, csb)
                                    nc.vector.tensor_add(rv[:, 1, :],
                                                         t1[:], t2[:])
                                nc.scalar.dma_start(
                                    dstl[b_i][ds(srow, 128), :], rp[:])

            # ========== phase 2: attention (+ wo transpose in gaps) ==========
            with tc.tile_pool(name="at", bufs=2) as at, \
                 tc.tile_pool(name="att", bufs=3) as att, \
                 tc.tile_pool(name="ps_st", bufs=3, space="PSUM") as ps_st, \
                 tc.tile_pool(name="ps_ac", bufs=1, space="PSUM") as ps_ac:
                def _bh_loads(b_i, h):
                    qTb = at.tile([128, S], BF, tag="qTb", name="qTb")
                    nc.sync.dma_start_transpose(
                        qTb[:], qn_ds[b_i][:, ts(h, HD)])
                    kTb = at.tile([128, S], BF, tag="kTb", name="kTb")
                    nc.sync.dma_start_transpose(
                        kTb[:], kn_ds[b_i][:, ts(h, HD)])
                    vb2 = at.tile([128, S // 128, HD], BF, tag="vb2",
                                  name="vb2")
                    nc.scalar.dma_start(
                        vb2[:],
                        v_ds[b_i][:, ts(h, HD)].rearrange(
                            "(kt p) d -> p kt d", p=128))
                    return qTb, kTb, vb2

                cur = _bh_loads(0, 0)
                for bh in range(B * H_LOC):
                    b_i, h = divmod(bh, H_LOC)
                    if True:
                        nxt = (_bh_loads(*divmod(bh + 1, H_LOC))
                               if bh + 1 < B * H_LOC else None)
                        qTb, kTb, vb2 = cur
                        for qc in range(NQC):
                            nkt = (qc + 1) * 4
                            stb = att.tile([128, S // 128, 512], BF, tag="stb",
                                           bufs=2)
                            for kt in range(nkt):
                                sps = ps_st.tile([128, 512], F32, tag="sps")
                                nc.tensor.matmul(sps[:],
                                                 lhsT=kTb[:, ts(kt, 128)],
                                                 rhs=qTb[:, ts(qc, 512)],
                                                 start=True, stop=True)
                                if kt // 4 == qc:
                                    off = (kt % 4) * 128
                                    if off > 0:
                                        nc.vector.memset(
                                            stb[:, kt, ds(0, off)], 0.0)
                                    sd = att.tile([128, 128], F32, tag="sd")
                                    nc.vector.scalar_tensor_tensor(
                                        sd[:], sps[:, ds(off, 128)], SCALE,
                                        maskT[:, kt, :], op0=MULT, op1=ADD)
                                    nc.scalar.activation(
                                        stb[:, kt, ds(off, 128)], sd[:], EXP)
                                    if off + 128 < 512:
                                        nc.scalar.activation(
                                            stb[:, kt,
                                                ds(off + 128, 384 - off)],
                                            sps[:, ds(off + 128, 384 - off)],
                                            EXP, scale=SCALE)
                                else:
                                    nc.scalar.activation(stb[:, kt, :], sps[:],
                                                         EXP, scale=SCALE)
                            # adapter scores [AL, 512]
                            spa = ps_st.tile([128, 512], F32, tag="sps")
                            nc.tensor.matmul(spa[:AL, :], lhsT=a_kT[:, h, :],
                                             rhs=qTb[:, ts(qc, 512)],
                                             start=True, stop=True)
                            pab = att.tile([AL, 512], BF, tag="pab")
                            nc.scalar.activation(pab[:], spa[:AL, :], EXP,
                                                 scale=SCALE)
                            # column sums via ones-matmul
                            s_ps = ps_ac.tile([1, 512], F32, tag="s_ps")
                            sa_ps = ps_ac.tile([1, 512], F32, tag="sa_ps")
                            for kt in range(nkt):
                                nc.tensor.matmul(s_ps[:], lhsT=ones[:, 0:1],
                                                 rhs=stb[:, kt, :],
                                                 start=(kt == 0),
                                                 stop=(kt == nkt - 1))
                            nc.tensor.matmul(sa_ps[:], lhsT=ones[:AL, 0:1],
                                             rhs=pab[:], start=True, stop=True)
                            # PV accumulation: oT [128 d, 512 q]
                            o_ps = ps_ac.tile([128, 512], F32, tag="o_ps", bufs=2)
                            for kt in range(nkt):
                                nc.tensor.matmul(o_ps[:], lhsT=vb2[:, kt, :],
                                                 rhs=stb[:, kt, :],
                                                 start=(kt == 0),
                                                 stop=(kt == nkt - 1))
                            oa_ps = ps_ac.tile([128, 512], F32, tag="oa_ps")
                            nc.tensor.matmul(oa_ps[:], lhsT=a_v[:, h, :],
                                             rhs=pab[:], start=True, stop=True)
                            # combine: o = o_main/s_main + tanh(g)*oa/s_adapt
                            sb2 = att.tile([1, 512], BF, tag="sb2")
                            nc.vector.tensor_copy(sb2[:], s_ps[:])
                            sb2a = att.tile([1, 512], BF, tag="sb2a")
                            nc.vector.tensor_copy(sb2a[:], sa_ps[:])
                            bc_ps = ps_st.tile([128, 512], F32, tag="sps")
                            nc.tensor.matmul(bc_ps[:], lhsT=ones[0:1, :],
                                             rhs=sb2[:], start=True, stop=True)
                            bca_ps = ps_st.tile([128, 512], F32, tag="sps")
                            nc.tensor.matmul(bca_ps[:], lhsT=ones[0:1, :],
                                             rhs=sb2a[:], start=True, stop=True)
                            rb = att.tile([128, 512], F32, tag="rb")
                            nc.vector.reciprocal_approx_fast(rb[:], bc_ps[:])
                            rba = att.tile([128, 512], F32, tag="rba")
                            nc.vector.reciprocal_approx_fast(rba[:], bca_ps[:])
                            t3 = att.tile([128, 512], F32, tag="t3")
                            nc.vector.tensor_mul(t3[:], o_ps[:], rb[:])
                            t4 = att.tile([128, 512], F32, tag="t4")
                            nc.vector.scalar_tensor_tensor(
                                t4[:], rba[:], g_sb[:, ds(h, 1)], oa_ps[:],
                                op0=MULT, op1=MULT)
                            ob = att.tile([128, 512], BF, tag="ob")
                            nc.vector.tensor_add(ob[:], t3[:], t4[:])
                            nc.scalar.dma_start(
                                oT_d[ts(h, HD),
                                     ds(b_i * S + qc * 512, 512)], ob[:])
                        # wo transpose chunks slotted into attention downtime
                        if bh >= 3:
                            for et in range((bh - 3) * 7,
                                            min(32, (bh - 3) * 7 + 7)):
                                wot_b = at.tile([128, D], BF, tag="wot_b",
                                                name="wot_b")
                                nc.sync.dma_start_transpose(
                                    wot_b[:], wob_d[:, ts(et, 128)])
                                nc.sync.dma_start(woT_d[ts(et, 128), :],
                                                    wot_b[:])
                        cur = nxt

            # ================= phase 3: AllToAll + wo =================
            for j in range(N_CORES):
                nc.scalar.dma_start(a2a_in[j], oT_d[:, ds(j * TPC, TPC)])
            nc.gpsimd.collective_compute(
                "AllToAll", mybir.AluOpType.bypass,
                replica_groups=[list(range(N_CORES))],
                ins=[a2a_in.opt()], outs=[a2a_out.opt()])
            with tc.tile_pool(name="wo_sb", bufs=3) as wsb, \
                 tc.tile_pool(name="wo_ps", bufs=1, space="PSUM") as wps, \
                 tc.tile_pool(name="of", bufs=1) as ofp:
                oTf = ofp.tile([128, NK, TPC], BF, tag="oTf")
                for sc in range(N_CORES):
                    nc.scalar.dma_start(
                        oTf[:, ds(sc * H_LOC, H_LOC), :],
                        a2a_out[sc].rearrange("(g p) t -> p g t", p=128))
                # 4 passes over d (1024 cols each); 8 psum banks = 4 tt x 2 d2
                for dp in range(4):
                    yps = [wps.tile([128, 512], F32, tag=f"yp{i}",
                                    name=f"yp{i}") for i in range(8)]
                    for et in range(NK):
                        wot = wsb.tile([128, 1024], BF, tag="wot")
                        nc.scalar.dma_start(
                            wot[:], woT_d[ts(et, 128), ts(dp, 1024)])
                        for tt in range(TPC // 128):
                            for d2 in range(2):
                                nc.tensor.matmul(
                                    yps[tt * 2 + d2][:],
                                    lhsT=oTf[:, et, ts(tt, 128)],
                                    rhs=wot[:, ts(d2, 512)],
                                    start=(et == 0), stop=(et == NK - 1))
                    for tt in range(TPC // 128):
                        for d2 in range(2):
                            yb = wsb.tile([128, 512], F32, tag="yb")
                            nc.vector.tensor_copy(yb[:], yps[tt * 2 + d2][:])
                            nc.scalar.dma_start(
                                out.ap()[ts(tt, 128),
                                         ds(dp * 1024 + d2 * 512, 512)],
                                yb[:])
    nc.compile()
    return nc


_NC_CACHE = None


def kernel(x, wq, wk, wv, wo, gate, adapter, freqs_cos, freqs_sin, mask,
           start_pos=0, **_unused):
    global _NC_CACHE
    if _NC_CACHE is None:
        _NC_CACHE = build()
    nc = _NC_CACHE
    xf = np.ascontiguousarray(np.asarray(x, np.float32).reshape(TOK, D))
    g = np.asarray(gate, np.float32).reshape(H)
    in_maps = []
    for r in range(N_CORES):
        sl = slice(r * CH, (r + 1) * CH)
        in_maps.append({
            "x": xf,
            "wq": np.ascontiguousarray(np.asarray(wq, np.float32)[sl]),
            "wk": np.ascontiguousarray(np.asarray(wk, np.float32)[sl]),
            "wv": np.ascontiguousarray(np.asarray(wv, np.float32)[sl]),
            "wo": np.ascontiguousarray(np.asarray(wo, np.float32)),
            "gate": np.ascontiguousarray(
                g[r * H_LOC:(r + 1) * H_LOC].reshape(1, H_LOC)),
            "adapter": np.ascontiguousarray(
                np.asarray(adapter, np.float32).reshape(AL, D)),
            "fcos": np.ascontiguousarray(np.asarray(freqs_cos, np.float32)),
            "fsin": np.ascontiguousarray(np.asarray(freqs_sin, np.float32)),
            "mask": np.ascontiguousarray(
                np.asarray(mask, np.float32).reshape(S, S)),
        })
    res = bass_utils.run_bass_kernel_spmd(nc, in_maps,
                                          core_ids=list(range(N_CORES)))
    y = np.concatenate([res.results[r]["out"] for r in range(N_CORES)], axis=0)
    return y.reshape(B, S, D)


if __name__ == "__main__":
    nc = build()
    print("compiled ok, instrs:",
          sum(len(bb.instructions) for f in nc.m.functions for bb in f.blocks))



# revision 12
# speedup vs baseline: 1.3129x; 1.3129x over previous
"""Distributed Trainium2 kernel for the gated-adapter attention module.

Head-parallel tensor parallelism over 8 NeuronCores (4 heads each).
Weights are host-packed (transposed, bf16, RoPE-pair-permuted for q/k) so
the device only streams x in f32, computes QKV with 512-wide bf16
matmuls, applies RoPE on contiguous 64-lane halves, runs flash-style
causal attention per head with scores held transposed (keys on
partitions), and finishes with an AllToAll (head-sharded -> token-
sharded) followed by the full wo projection per 512-token slice.
Softmax column sums use a DVE accumulation tree plus one ones-matmul;
denominator broadcasts ride the (otherwise idle) GPSIMD engine.
"""

import sys

sys.path.insert(0, "/opt/trn_rl_repo")

import numpy as np
import ml_dtypes

import concourse.bass as bass
import concourse.mybir as mybir
import concourse.tile as tile
from concourse import bacc, bass_utils
from concourse.bass import ds, ts
from concourse.masks import make_identity

N_CORES = 8
B, S, D = 2, 2048, 4096
H = 32
HD = 128                      # head dim
H_LOC = H // N_CORES          # 4 heads per core
CH = H_LOC * HD               # 512 local channels
TOK = B * S                   # 4096 tokens
NK = D // 128                 # 32 contraction tiles
AL = 10                       # adapter length
TPC = TOK // N_CORES          # 512 tokens per core after AllToAll
NQC = S // 512                # 4 query chunks per sequence
NCHB = S // 128               # 16 token chunks per batch
SCALE = 1.0 / float(np.sqrt(HD))
BF = mybir.dt.bfloat16
F32 = mybir.dt.float32
EXP = mybir.ActivationFunctionType.Exp
MULT = mybir.AluOpType.mult
ADD = mybir.AluOpType.add


def build():
    nc = bacc.Bacc("TRN2", target_bir_lowering=False, debug=False,
                   num_devices=N_CORES)
    x = nc.dram_tensor("x", [TOK, D], F32, kind="ExternalInput")
    wt3 = nc.dram_tensor("wt3", [3, D, CH], BF, kind="ExternalInput")
    wot = nc.dram_tensor("wot", [D, D], BF, kind="ExternalInput")
    maskd = nc.dram_tensor("maskd", [NCHB, 128, 128], BF, kind="ExternalInput")
    fc = nc.dram_tensor("fc", [S, HD // 2], BF, kind="ExternalInput")
    fs = nc.dram_tensor("fs", [S, HD // 2], BF, kind="ExternalInput")
    adT = nc.dram_tensor("adT", [D, AL], BF, kind="ExternalInput")
    gth = nc.dram_tensor("gth", [1, H_LOC], F32, kind="ExternalInput")
    out = nc.dram_tensor("out", [TPC, D], F32, kind="ExternalOutput")

    with tile.TileContext(nc) as tc:
        with tc.tile_pool(name="dram", bufs=1, space="DRAM") as dram, \
             tc.tile_pool(name="persist", bufs=1) as persist:
            # q/k spilled transposed per (b, h): contiguous [128, S] reads
            qkT_d = dram.tile([2, B * H_LOC, HD, S], BF, tag="qkT_d")
            v_d = [dram.tile([S, CH], BF, tag=f"v{b}", name=f"v{b}")
                   for b in range(B)]
            a2a_in = dram.tile([N_CORES, CH, TPC], BF, tag="a2a_in")
            a2a_out = dram.tile([N_CORES, CH, TPC], BF, tag="a2a_out")

            ident = persist.tile([128, 128], BF, tag="ident")
            make_identity(nc, ident[:])
            ones01 = persist.tile([128, 1], BF, tag="ones01")
            nc.vector.memset(ones01[:], 1.0)
            g_sb = persist.tile([128, H_LOC], F32, tag="g_sb")
            nc.scalar.dma_start(g_sb[:], gth.ap().partition_broadcast(128))
            cs_sb = persist.tile([128, NCHB, HD // 2], BF, tag="cs_sb")
            nc.scalar.dma_start(
                cs_sb[:], fc.ap().rearrange("(pb p) f -> p pb f", p=128))
            sn_sb = persist.tile([128, NCHB, HD // 2], BF, tag="sn_sb")
            nc.scalar.dma_start(
                sn_sb[:], fs.ap().rearrange("(pb p) f -> p pb f", p=128))
            maskT = persist.tile([128, NCHB, 128], BF, tag="maskT")
            nc.scalar.dma_start(
                maskT[:], maskd.ap().rearrange("d p q -> p d q"))
            aT = persist.tile([128, NK, AL], BF, tag="aT")
            nc.scalar.dma_start(
                aT[:], adT.ap().rearrange("(k p) a -> p k a", p=128))
            a_kT = persist.tile([128, H_LOC, AL], BF, tag="a_kT")
            a_v = persist.tile([AL, CH], BF, tag="a_v")

            with tc.tile_pool(name="wtp", bufs=1) as wtp, \
                 tc.tile_pool(name="run", bufs=2) as run, \
                 tc.tile_pool(name="att", bufs=2) as att, \
                 tc.tile_pool(name="stp", bufs=4) as stp, \
                 tc.tile_pool(name="pp_ps", bufs=2, space="PSUM") as pp_ps, \
                 tc.tile_pool(name="tp_ps", bufs=2, space="PSUM") as tp_ps, \
                 tc.tile_pool(name="sc_ps", bufs=2, space="PSUM") as sc_ps, \
                 tc.tile_pool(name="po_ps", bufs=1, space="PSUM") as po_ps:
                wT = wtp.tile([128, 3, NK, CH], BF, tag="wT")
                nc.sync.dma_start(
                    wT[:], wt3.ap().rearrange("t (k p) c -> p t k c", p=128))

                # adapter projections: a_kT per head, a_v
                for h in range(H_LOC):
                    pk = sc_ps.tile([128, 512], F32, tag="sc")
                    for dt in range(NK):
                        nc.tensor.matmul(pk[:, :AL],
                                         lhsT=wT[:, 1, dt, ts(h, HD)],
                                         rhs=aT[:, dt, :], start=(dt == 0),
                                         stop=(dt == NK - 1))
                    nc.vector.tensor_copy(a_kT[:, h, :], pk[:, :AL])
                pv = sc_ps.tile([128, 512], F32, tag="sc")
                for dt in range(NK):
                    nc.tensor.matmul(pv[:AL, :], lhsT=aT[:, dt, :],
                                     rhs=wT[:, 2, dt, :], start=(dt == 0),
                                     stop=(dt == NK - 1))
                nc.vector.tensor_copy(a_v[:], pv[:AL, :])

                def emit_chunk(b_i, c16):
                    """QKV + RoPE + spills for one 128-token chunk."""
                    tstr = b_i * NCHB + c16
                    xT = run.tile([128, NK, 128], BF, tag="xT")
                    for hf in range(4):
                        xf = run.tile([128, D // 4], F32, tag="xf")
                        nc.sync.dma_start(
                            xf[:], x.ap()[ts(tstr, 128), ts(hf, D // 4)])
                        xb = run.tile([128, D // 4], BF, tag="xb")
                        nc.vector.tensor_copy(xb[:], xf[:])
                        for q4 in range(2):
                            tps = tp_ps.tile([128, 512], BF, tag="tp")
                            for j in range(4):
                                nc.tensor.transpose(
                                    tps[:, ts(j, 128)],
                                    xb[:, ts(q4 * 4 + j, 128)], ident[:])
                            nc.vector.tensor_copy(
                                xT[:, ds(hf * 8 + q4 * 4, 4), :].rearrange(
                                    "p a b -> p (a b)"), tps[:])
                    csb = cs_sb[:, c16, None, :].broadcast_to([128, H_LOC, 64])
                    snb = sn_sb[:, c16, None, :].broadcast_to([128, H_LOC, 64])
                    for p_i in range(3):
                        pp = pp_ps.tile([128, CH], F32, tag="pp")
                        for dt in range(NK):
                            nc.tensor.matmul(pp[:], lhsT=xT[:, dt, :],
                                             rhs=wT[:, p_i, dt, :],
                                             start=(dt == 0),
                                             stop=(dt == NK - 1))
                        if p_i == 2:
                            vb = run.tile([128, CH], BF, tag="vb")
                            nc.vector.tensor_copy(vb[:], pp[:])
                            nc.sync.dma_start(
                                v_d[b_i][ts(c16, 128), :], vb[:])
                            return
                        ppv = pp[:].rearrange("p (h i) -> p h i", h=H_LOC)
                        pa, pb = ppv[:, :, 0:64], ppv[:, :, 64:128]
                        t1 = run.tile([128, H_LOC, 64], F32, tag="t1")
                        t2 = run.tile([128, H_LOC, 64], F32, tag="t2")
                        rq = run.tile([128, CH], BF, tag=f"rq{p_i}",
                                      name=f"rq{p_i}")
                        rqv = rq[:].rearrange("p (h i) -> p h i", h=H_LOC)
                        nc.vector.tensor_mul(t1[:], pa, csb)
                        nc.vector.tensor_mul(t2[:], pb, snb)
                        nc.vector.tensor_sub(rqv[:, :, 0:64], t1[:], t2[:])
                        nc.vector.tensor_mul(t1[:], pa, snb)
                        nc.vector.tensor_mul(t2[:], pb, csb)
                        nc.vector.tensor_add(rqv[:, :, 64:128], t1[:], t2[:])
                        tps = tp_ps.tile([128, 512], BF, tag="tp")
                        for h in range(H_LOC):
                            nc.tensor.transpose(
                                tps[:, ts(h, 128)], rq[:, ts(h, HD)], ident[:])
                        stg = run.tile([128, 512], BF, tag=f"st{p_i}",
                                       name=f"st{p_i}")
                        nc.vector.tensor_copy(stg[:], tps[:])
                        nc.sync.dma_start(
                            qkT_d[p_i, ds(b_i * H_LOC, H_LOC), :,
                                  ts(c16, 128)].rearrange("h p t -> p h t"),
                            stg[:].rearrange("p (h t) -> p h t", h=H_LOC))

                def emit_attn(b_i, h):
                    """Flash attention for one (batch, local head)."""
                    bh = b_i * H_LOC + h
                    qTb = att.tile([128, S], BF, tag="qTb")
                    nc.scalar.dma_start(qTb[:], qkT_d[0, bh])
                    kTb = att.tile([128, S], BF, tag="kTb")
                    nc.scalar.dma_start(kTb[:], qkT_d[1, bh])
                    vb2 = att.tile([128, NCHB, HD], BF, tag="vb2")
                    nc.scalar.dma_start(
                        vb2[:],
                        v_d[b_i][:, ts(h, HD)].rearrange(
                            "(kt p) d -> p kt d", p=128))
                    for qc in range(NQC):
                        nkt = (qc + 1) * 4
                        o_ps = po_ps.tile([128, 512], F32, tag="o")
                        acc = att.tile([128, 512], F32, tag="acc")
                        accb = att.tile([128, 512], BF, tag="accb")
                        for kt in range(nkt):
                            sps = sc_ps.tile([128, 512], F32, tag="sc")
                            nc.tensor.matmul(sps[:], lhsT=kTb[:, ts(kt, 128)],
                                             rhs=qTb[:, ts(qc, 512)],
                                             start=True, stop=True)
                            stb = stp.tile([128, 512], BF, tag="stb")
                            if kt // 4 == qc:
                                off = (kt % 4) * 128
                                if off > 0:
                                    nc.vector.memset(stb[:, ds(0, off)], 0.0)
                                sd = stp.tile([128, 128], F32, tag="sd",
                                              bufs=2)
                                nc.vector.scalar_tensor_tensor(
                                    sd[:], sps[:, ds(off, 128)], SCALE,
                                    maskT[:, kt, :], op0=MULT, op1=ADD)
                                nc.scalar.activation(
                                    stb[:, ds(off, 128)], sd[:], EXP)
                                if off + 128 < 512:
                                    nc.scalar.activation(
                                        stb[:, ds(off + 128, 384 - off)],
                                        sps[:, ds(off + 128, 384 - off)],
                                        EXP, scale=SCALE)
                            else:
                                nc.scalar.activation(stb[:], sps[:], EXP,
                                                     scale=SCALE)
                            nc.tensor.matmul(o_ps[:], lhsT=vb2[:, kt, :],
                                             rhs=stb[:], start=(kt == 0),
                                             stop=(kt == nkt - 1))
                            if kt == 0:
                                nc.vector.tensor_copy(acc[:], stb[:])
                            elif kt < nkt - 1:
                                nc.vector.tensor_add(acc[:], acc[:], stb[:])
                            else:
                                nc.vector.tensor_add(accb[:], acc[:], stb[:])
                        # adapter cross-attention (own softmax)
                        spa = sc_ps.tile([128, 512], F32, tag="sc")
                        nc.tensor.matmul(spa[:AL, :], lhsT=a_kT[:, h, :],
                                         rhs=qTb[:, ts(qc, 512)],
                                         start=True, stop=True)
                        pab = stp.tile([AL, 512], BF, tag="pab", bufs=2)
                        nc.scalar.activation(pab[:], spa[:AL, :], EXP,
                                             scale=SCALE)
                        oa_ps = po_ps.tile([128, 512], F32, tag="oa")
                        nc.tensor.matmul(oa_ps[:], lhsT=a_v[:, ts(h, HD)],
                                         rhs=pab[:], start=True, stop=True)
                        # denominators: ones-matmul column sums
                        s2 = sc_ps.tile([128, 512], F32, tag="sc")
                        nc.tensor.matmul(s2[0:1, :], lhsT=ones01[:, 0:1],
                                         rhs=accb[:], start=True, stop=True)
                        sa2 = sc_ps.tile([128, 512], F32, tag="sc")
                        nc.tensor.matmul(sa2[0:1, :], lhsT=ones01[:AL, 0:1],
                                         rhs=pab[:], start=True, stop=True)
                        den = att.tile([1, 1024], F32, tag="den", bufs=1)
                        nc.vector.tensor_copy(den[:, 0:512], s2[0:1, :])
                        nc.vector.tensor_copy(den[:, 512:1024], sa2[0:1, :])
                        rden = att.tile([1, 1024], F32, tag="rden", bufs=1)
                        nc.vector.reciprocal_approx_fast(rden[:], den[:])
                        rbc = att.tile([128, 1024], F32, tag="rbc", bufs=1)
                        nc.gpsimd.partition_broadcast(rbc[:], rden[:])
                        t3 = att.tile([128, 512], F32, tag="t3", bufs=1)
                        nc.vector.tensor_mul(t3[:], o_ps[:], rbc[:, 0:512])
                        t4 = att.tile([128, 512], F32, tag="t4", bufs=1)
                        nc.vector.scalar_tensor_tensor(
                            t4[:], oa_ps[:], g_sb[:, ds(h, 1)],
                            rbc[:, 512:1024], op0=MULT, op1=MULT)
                        ob = att.tile([128, 512], BF, tag="ob")
                        nc.vector.tensor_add(ob[:], t3[:], t4[:])
                        nc.scalar.dma_start(
                            a2a_in[b_i * NQC + qc, ts(h, HD), :], ob[:])

                # batch 0 QKV, then batch 1 QKV interleaved with batch 0
                # attention, then batch 1 attention
                for c16 in range(NCHB):
                    emit_chunk(0, c16)
                for grp in range(4):
                    for c16 in range(grp * 4, grp * 4 + 4):
                        emit_chunk(1, c16)
                    emit_attn(0, grp)
                for h in range(H_LOC):
                    emit_attn(1, h)

            # ================= AllToAll + wo =================
            nc.gpsimd.collective_compute(
                "AllToAll", mybir.AluOpType.bypass,
                replica_groups=[list(range(N_CORES))],
                ins=[a2a_in.opt()], outs=[a2a_out.opt()])
            with tc.tile_pool(name="wsb", bufs=4) as wsb, \
                 tc.tile_pool(name="ofp", bufs=1) as ofp, \
                 tc.tile_pool(name="wps", bufs=1, space="PSUM") as wps:
                oTf = ofp.tile([128, NK, TPC], BF, tag="oTf")
                for sc in range(N_CORES):
                    nc.scalar.dma_start(
                        oTf[:, ds(sc * H_LOC, H_LOC), :],
                        a2a_out[sc].rearrange("(g p) t -> p g t", p=128))
                # 4 passes over d (1024 cols each); 8 psum banks = 4 tt x 2 d2
                for dp in range(4):
                    yps = [wps.tile([128, 512], F32, tag=f"yp{i}",
                                    name=f"yp{i}") for i in range(8)]
                    for et in range(NK):
                        wot_t = wsb.tile([128, 1024], BF, tag="wot_t")
                        nc.sync.dma_start(
                            wot_t[:], wot.ap()[ts(et, 128), ts(dp, 1024)])
                        for tt in range(TPC // 128):
                            for d2 in range(2):
                                nc.tensor.matmul(
                                    yps[tt * 2 + d2][:],
                                    lhsT=oTf[:, et, ts(tt, 128)],
                                    rhs=wot_t[:, ts(d2, 512)],
                                    start=(et == 0), stop=(et == NK - 1))
                    for tt in range(TPC // 128):
                        for d2 in range(2):
                            yb = wsb.tile([128, 512], F32, tag="yb")
                            nc.vector.tensor_copy(yb[:], yps[tt * 2 + d2][:])
                            nc.scalar.dma_start(
                                out.ap()[ts(tt, 128),
                                         ds(dp * 1024 + d2 * 512, 512)],
                                yb[:])
    nc.compile()
    return nc


_NC_CACHE = None
_ROPE_PERM = np.concatenate(
    [np.arange(0, HD, 2), np.arange(1, HD, 2)])  # pair halves within a head


def _pack_inputs(x, wq, wk, wv, wo, gate, adapter, freqs_cos, freqs_sin,
                 mask):
    bf = ml_dtypes.bfloat16
    xf = np.ascontiguousarray(np.asarray(x, np.float32).reshape(TOK, D))
    mk = np.asarray(mask, np.float32).reshape(S, S)
    maskd = np.ascontiguousarray(np.stack(
        [mk[d * 128:(d + 1) * 128, d * 128:(d + 1) * 128].T
         for d in range(NCHB)])).astype(bf)
    wot = np.ascontiguousarray(np.asarray(wo, np.float32).T).astype(bf)
    adT = np.ascontiguousarray(
        np.asarray(adapter, np.float32).reshape(AL, D).T).astype(bf)
    gt = np.tanh(np.asarray(gate, np.float32).reshape(H))
    fc = np.ascontiguousarray(np.asarray(freqs_cos, np.float32)).astype(bf)
    fs = np.ascontiguousarray(np.asarray(freqs_sin, np.float32)).astype(bf)
    perm_full = (np.arange(CH).reshape(H_LOC, HD) * 0
                 + _ROPE_PERM[None, :]
                 + (np.arange(H_LOC) * HD)[:, None]).reshape(CH)
    in_maps = []
    for r in range(N_CORES):
        sl = slice(r * CH, (r + 1) * CH)
        wq_p = np.asarray(wq, np.float32)[sl][perm_full]
        wk_p = np.asarray(wk, np.float32)[sl][perm_full]
        wv_s = np.asarray(wv, np.float32)[sl]
        wt3 = np.ascontiguousarray(
            np.stack([wq_p.T, wk_p.T, wv_s.T])).astype(bf)
        in_maps.append({
            "x": xf,
            "wt3": wt3,
            "wot": wot,
            "maskd": maskd,
            "fc": fc,
            "fs": fs,
            "adT": adT,
            "gth": np.ascontiguousarray(
                gt[r * H_LOC:(r + 1) * H_LOC].reshape(1, H_LOC)),
        })
    return in_maps


def kernel(x, wq, wk, wv, wo, gate, adapter, freqs_cos, freqs_sin, mask,
           start_pos=0, **_unused):
    global _NC_CACHE
    if _NC_CACHE is None:
        _NC_CACHE = build()
    nc = _NC_CACHE
    in_maps = _pack_inputs(x, wq, wk, wv, wo, gate, adapter, freqs_cos,
                           freqs_sin, mask)
    res = bass_utils.run_bass_kernel_spmd(nc, in_maps,
                                          core_ids=list(range(N_CORES)))
    y = np.concatenate([res.results[r]["out"] for r in range(N_CORES)], axis=0)
    return y.reshape(B, S, D)


if __name__ == "__main__":
    nc = build()
    print("compiled ok, instrs:",
          sum(len(bb.instructions) for f in nc.m.functions for bb in f.blocks))


# revision 13
# speedup vs baseline: 1.3423x; 1.0224x over previous
"""Distributed Trainium2 kernel for the gated-adapter attention module.

Head-parallel tensor parallelism over 8 NeuronCores (4 heads each).
Weights are host-packed (transposed, bf16, RoPE-pair-permuted for q/k) so
the device only streams x in f32, computes QKV with 512-wide bf16
matmuls, applies RoPE on contiguous 64-lane halves, runs flash-style
causal attention per head with scores held transposed (keys on
partitions), and finishes with a split AllToAll (head-sharded ->
token-sharded, two head-halves so the collective overlaps the attention
tail) followed by the full wo projection per 512-token slice.  Batch-1
QKV is interleaved with batch-0 attention; softmax column sums use a
DVE tree in the tensor-bound region and ones-matmuls in the
attention-only tail; PSUM->SBUF copies ride the scalar engine.
"""

import sys

sys.path.insert(0, "/opt/trn_rl_repo")

import numpy as np
import ml_dtypes

import concourse.bass as bass
import concourse.mybir as mybir
import concourse.tile as tile
from concourse import bacc, bass_utils
from concourse.bass import ds, ts
from concourse.masks import make_identity

N_CORES = 8
B, S, D = 2, 2048, 4096
H = 32
HD = 128                      # head dim
H_LOC = H // N_CORES          # 4 heads per core
CH = H_LOC * HD               # 512 local channels
TOK = B * S                   # 4096 tokens
NK = D // 128                 # 32 contraction tiles
AL = 10                       # adapter length
TPC = TOK // N_CORES          # 512 tokens per core after AllToAll
NQC = S // 512                # 4 query chunks per sequence
NCHB = S // 128               # 16 token chunks per batch
SCALE = 1.0 / float(np.sqrt(HD))
BF = mybir.dt.bfloat16
F32 = mybir.dt.float32
EXP = mybir.ActivationFunctionType.Exp
COPY = mybir.ActivationFunctionType.Copy
MULT = mybir.AluOpType.mult
ADD = mybir.AluOpType.add


def build():
    nc = bacc.Bacc("TRN2", target_bir_lowering=False, debug=False,
                   num_devices=N_CORES)
    x = nc.dram_tensor("x", [TOK, D], F32, kind="ExternalInput")
    wt3 = nc.dram_tensor("wt3", [3, D, CH], BF, kind="ExternalInput")
    wot = nc.dram_tensor("wot", [D, D], BF, kind="ExternalInput")
    maskd = nc.dram_tensor("maskd", [NCHB, 128, 128], BF, kind="ExternalInput")
    fc = nc.dram_tensor("fc", [S, HD // 2], BF, kind="ExternalInput")
    fs = nc.dram_tensor("fs", [S, HD // 2], BF, kind="ExternalInput")
    adT = nc.dram_tensor("adT", [D, AL], BF, kind="ExternalInput")
    gth = nc.dram_tensor("gth", [1, H_LOC], F32, kind="ExternalInput")
    out = nc.dram_tensor("out", [TPC, D], F32, kind="ExternalOutput")

    with tile.TileContext(nc) as tc:
        with tc.tile_pool(name="dram", bufs=1, space="DRAM") as dram, \
             tc.tile_pool(name="persist", bufs=1) as persist:
            # q/k spilled transposed per (b, h): contiguous [128, S] reads
            qkT_d = dram.tile([2, B * H_LOC, HD, S], BF, tag="qkT_d")
            v_d = [dram.tile([S, CH], BF, tag=f"v{b}", name=f"v{b}")
                   for b in range(B)]
            # AllToAll split into two head-halves (h0/h1 vs h2/h3) so the
            # first collective overlaps the attention tail.
            a2a_in = [dram.tile([N_CORES, CH // 2, TPC], BF, tag=f"ai{i}",
                                name=f"ai{i}") for i in range(2)]
            a2a_out = [dram.tile([N_CORES, CH // 2, TPC], BF, tag=f"ao{i}",
                                 name=f"ao{i}") for i in range(2)]

            ident = persist.tile([128, 128], BF, tag="ident")
            make_identity(nc, ident[:])
            ones01 = persist.tile([128, 1], BF, tag="ones01")
            nc.vector.memset(ones01[:], 1.0)
            g_sb = persist.tile([128, H_LOC], F32, tag="g_sb")
            nc.scalar.dma_start(g_sb[:], gth.ap().partition_broadcast(128))
            cs_sb = persist.tile([128, NCHB, HD // 2], BF, tag="cs_sb")
            nc.scalar.dma_start(
                cs_sb[:], fc.ap().rearrange("(pb p) f -> p pb f", p=128))
            sn_sb = persist.tile([128, NCHB, HD // 2], BF, tag="sn_sb")
            nc.scalar.dma_start(
                sn_sb[:], fs.ap().rearrange("(pb p) f -> p pb f", p=128))
            maskT = persist.tile([128, NCHB, 128], BF, tag="maskT")
            nc.scalar.dma_start(
                maskT[:], maskd.ap().rearrange("d p q -> p d q"))
            aT = persist.tile([128, NK, AL], BF, tag="aT")
            nc.scalar.dma_start(
                aT[:], adT.ap().rearrange("(k p) a -> p k a", p=128))
            a_kT = persist.tile([128, H_LOC, AL], BF, tag="a_kT")
            a_v = persist.tile([AL, CH], BF, tag="a_v")

            def emit_attn(b_i, h, att, stp, sc_ps, po_ps, colsum_mm):
                """Flash attention for one (batch, local head)."""
                bh = b_i * H_LOC + h
                qTb = att.tile([128, S], BF, tag="qTb")
                nc.sync.dma_start(qTb[:], qkT_d[0, bh])
                kTb = att.tile([128, S], BF, tag="kTb")
                nc.sync.dma_start(kTb[:], qkT_d[1, bh])
                vb2 = att.tile([128, NCHB, HD], BF, tag="vb2")
                nc.sync.dma_start(
                    vb2[:],
                    v_d[b_i][:, ts(h, HD)].rearrange(
                        "(kt p) d -> p kt d", p=128))
                for qc in range(NQC):
                    nkt = (qc + 1) * 4
                    o_ps = po_ps.tile([128, 512], F32, tag="o")
                    if colsum_mm:
                        s_ps = po_ps.tile([128, 512], F32, tag="s")
                    else:
                        acc = att.tile([128, 512], F32, tag="acc")
                        accb = att.tile([128, 512], BF, tag="accb")
                    for kt in range(nkt):
                        sps = sc_ps.tile([128, 512], F32, tag="sc")
                        nc.tensor.matmul(sps[:], lhsT=kTb[:, ts(kt, 128)],
                                         rhs=qTb[:, ts(qc, 512)],
                                         start=True, stop=True)
                        stb = stp.tile([128, 512], BF, tag="stb")
                        if kt // 4 == qc:
                            off = (kt % 4) * 128
                            if off > 0:
                                nc.vector.memset(stb[:, ds(0, off)], 0.0)
                            sd = stp.tile([128, 128], F32, tag="sd", bufs=2)
                            nc.vector.scalar_tensor_tensor(
                                sd[:], sps[:, ds(off, 128)], SCALE,
                                maskT[:, kt, :], op0=MULT, op1=ADD)
                            nc.scalar.activation(
                                stb[:, ds(off, 128)], sd[:], EXP)
                            if off + 128 < 512:
                                nc.scalar.activation(
                                    stb[:, ds(off + 128, 384 - off)],
                                    sps[:, ds(off + 128, 384 - off)],
                                    EXP, scale=SCALE)
                        else:
                            nc.scalar.activation(stb[:], sps[:], EXP,
                                                 scale=SCALE)
                        nc.tensor.matmul(o_ps[:], lhsT=vb2[:, kt, :],
                                         rhs=stb[:], start=(kt == 0),
                                         stop=(kt == nkt - 1))
                        if colsum_mm:
                            nc.tensor.matmul(s_ps[0:1, :],
                                             lhsT=ones01[:, 0:1], rhs=stb[:],
                                             start=(kt == 0),
                                             stop=(kt == nkt - 1))
                        elif kt == 0:
                            nc.vector.tensor_copy(acc[:], stb[:])
                        elif kt < nkt - 1:
                            nc.vector.tensor_add(acc[:], acc[:], stb[:])
                        else:
                            nc.vector.tensor_add(accb[:], acc[:], stb[:])
                    # adapter cross-attention (own softmax)
                    spa = sc_ps.tile([128, 512], F32, tag="sc")
                    nc.tensor.matmul(spa[:AL, :], lhsT=a_kT[:, h, :],
                                     rhs=qTb[:, ts(qc, 512)],
                                     start=True, stop=True)
                    pab = stp.tile([AL, 512], BF, tag="pab", bufs=2)
                    nc.scalar.activation(pab[:], spa[:AL, :], EXP,
                                         scale=SCALE)
                    oa_ps = po_ps.tile([128, 512], F32, tag="oa")
                    nc.tensor.matmul(oa_ps[:], lhsT=a_v[:, ts(h, HD)],
                                     rhs=pab[:], start=True, stop=True)
                    # denominators: ones-matmul column sums
                    if not colsum_mm:
                        s_ps = sc_ps.tile([128, 512], F32, tag="sc")
                        nc.tensor.matmul(s_ps[0:1, :], lhsT=ones01[:, 0:1],
                                         rhs=accb[:], start=True, stop=True)
                    sa2 = sc_ps.tile([128, 512], F32, tag="sc")
                    nc.tensor.matmul(sa2[0:1, :], lhsT=ones01[:AL, 0:1],
                                     rhs=pab[:], start=True, stop=True)
                    den = att.tile([1, 1024], F32, tag="den", bufs=1)
                    nc.scalar.activation(den[:, 0:512], s_ps[0:1, :], COPY)
                    nc.scalar.activation(den[:, 512:1024], sa2[0:1, :], COPY)
                    dbc = att.tile([128, 1024], F32, tag="dbc", bufs=1)
                    nc.gpsimd.partition_broadcast(dbc[:], den[:])
                    rbc = att.tile([128, 1024], F32, tag="rbc", bufs=1)
                    nc.vector.reciprocal_approx_fast(rbc[:], dbc[:])
                    t3 = att.tile([128, 512], F32, tag="t3", bufs=1)
                    nc.vector.tensor_mul(t3[:], o_ps[:], rbc[:, 0:512])
                    t4 = att.tile([128, 512], F32, tag="t4", bufs=1)
                    nc.vector.scalar_tensor_tensor(
                        t4[:], oa_ps[:], g_sb[:, ds(h, 1)],
                        rbc[:, 512:1024], op0=MULT, op1=MULT)
                    ob = att.tile([128, 512], BF, tag="ob")
                    nc.vector.tensor_add(ob[:], t3[:], t4[:])
                    nc.sync.dma_start(
                        a2a_in[h // 2][b_i * NQC + qc,
                                       ts(h % 2, HD), :], ob[:])

            # ======== scope 1: QKV (both batches) + attention b0 ========
            with tc.tile_pool(name="wtp", bufs=1) as wtp, \
                 tc.tile_pool(name="run", bufs=2) as run, \
                 tc.tile_pool(name="att", bufs=2) as att, \
                 tc.tile_pool(name="stp", bufs=4) as stp, \
                 tc.tile_pool(name="pp_ps", bufs=2, space="PSUM") as pp_ps, \
                 tc.tile_pool(name="tp_ps", bufs=2, space="PSUM") as tp_ps, \
                 tc.tile_pool(name="sc_ps", bufs=2, space="PSUM") as sc_ps, \
                 tc.tile_pool(name="po_ps", bufs=1, space="PSUM") as po_ps:
                wT = wtp.tile([128, 3, NK, CH], BF, tag="wT")
                nc.scalar.dma_start(
                    wT[:], wt3.ap().rearrange("t (k p) c -> p t k c", p=128))

                # adapter projections: a_kT per head, a_v
                for h in range(H_LOC):
                    pk = sc_ps.tile([128, 512], F32, tag="sc")
                    for dt in range(NK):
                        nc.tensor.matmul(pk[:, :AL],
                                         lhsT=wT[:, 1, dt, ts(h, HD)],
                                         rhs=aT[:, dt, :], start=(dt == 0),
                                         stop=(dt == NK - 1))
                    nc.vector.tensor_copy(a_kT[:, h, :], pk[:, :AL])
                pv = sc_ps.tile([128, 512], F32, tag="sc")
                for dt in range(NK):
                    nc.tensor.matmul(pv[:AL, :], lhsT=aT[:, dt, :],
                                     rhs=wT[:, 2, dt, :], start=(dt == 0),
                                     stop=(dt == NK - 1))
                nc.vector.tensor_copy(a_v[:], pv[:AL, :])

                def emit_chunk(b_i, c16):
                    """QKV + RoPE + spills for one 128-token chunk."""
                    tstr = b_i * NCHB + c16
                    xT = run.tile([128, NK, 128], BF, tag="xT")
                    for hf in range(4):
                        xf = run.tile([128, D // 4], F32, tag="xf")
                        nc.sync.dma_start(
                            xf[:], x.ap()[ts(tstr, 128), ts(hf, D // 4)])
                        xb = run.tile([128, D // 4], BF, tag="xb")
                        nc.vector.tensor_copy(xb[:], xf[:])
                        tps = tp_ps.tile([128, 1024], BF, tag="tp")
                        for j in range(8):
                            nc.tensor.transpose(
                                tps[:, ts(j, 128)], xb[:, ts(j, 128)],
                                ident[:])
                        nc.scalar.activation(
                            xT[:, ds(hf * 8, 8), :].rearrange(
                                "p a b -> p (a b)"), tps[:], COPY)
                    csb = cs_sb[:, c16, None, :].broadcast_to([128, H_LOC, 64])
                    snb = sn_sb[:, c16, None, :].broadcast_to([128, H_LOC, 64])
                    for p_i in range(3):
                        pp = pp_ps.tile([128, CH], F32, tag="pp")
                        for dt in range(NK):
                            nc.tensor.matmul(pp[:], lhsT=xT[:, dt, :],
                                             rhs=wT[:, p_i, dt, :],
                                             start=(dt == 0),
                                             stop=(dt == NK - 1))
                        if p_i == 2:
                            vb = run.tile([128, CH], BF, tag="vb")
                            nc.scalar.activation(vb[:], pp[:], COPY)
                            nc.sync.dma_start(
                                v_d[b_i][ts(c16, 128), :], vb[:])
                            return
                        ppv = pp[:].rearrange("p (h i) -> p h i", h=H_LOC)
                        pa, pb = ppv[:, :, 0:64], ppv[:, :, 64:128]
                        t1 = run.tile([128, H_LOC, 64], F32, tag="t1")
                        t2 = run.tile([128, H_LOC, 64], F32, tag="t2")
                        rq = run.tile([128, CH], BF, tag=f"rq{p_i}",
                                      name=f"rq{p_i}")
                        rqv = rq[:].rearrange("p (h i) -> p h i", h=H_LOC)
                        nc.vector.tensor_mul(t1[:], pa, csb)
                        nc.vector.tensor_mul(t2[:], pb, snb)
                        nc.vector.tensor_sub(rqv[:, :, 0:64], t1[:], t2[:])
                        nc.vector.tensor_mul(t1[:], pa, snb)
                        nc.vector.tensor_mul(t2[:], pb, csb)
                        nc.vector.tensor_add(rqv[:, :, 64:128], t1[:], t2[:])
                        tps = tp_ps.tile([128, 1024], BF, tag="tp")
                        for h in range(H_LOC):
                            nc.tensor.transpose(
                                tps[:, ts(h, 128)], rq[:, ts(h, HD)], ident[:])
                        stg = run.tile([128, 512], BF, tag=f"st{p_i}",
                                       name=f"st{p_i}")
                        nc.scalar.activation(stg[:], tps[:, 0:512], COPY)
                        nc.sync.dma_start(
                            qkT_d[p_i, ds(b_i * H_LOC, H_LOC), :,
                                  ts(c16, 128)].rearrange("h p t -> p h t"),
                            stg[:].rearrange("p (h t) -> p h t", h=H_LOC))

                # batch 0 QKV, then batch 1 QKV interleaved with batch 0
                # attention
                for c16 in range(NCHB):
                    emit_chunk(0, c16)
                for grp in range(4):
                    for c16 in range(grp * 4, grp * 4 + 4):
                        emit_chunk(1, c16)
                    emit_attn(0, grp, att, stp, sc_ps, po_ps,
                              colsum_mm=False)

            # ======== scope 2: attention b1 tail + split AllToAll ========
            with tc.tile_pool(name="att2", bufs=2) as att2, \
                 tc.tile_pool(name="stp2", bufs=4) as stp2, \
                 tc.tile_pool(name="sc2_ps", bufs=4, space="PSUM") as sc2, \
                 tc.tile_pool(name="po2_ps", bufs=1, space="PSUM") as po2:
                emit_attn(1, 0, att2, stp2, sc2, po2, colsum_mm=True)
                emit_attn(1, 1, att2, stp2, sc2, po2, colsum_mm=True)
                nc.gpsimd.collective_compute(
                    "AllToAll", mybir.AluOpType.bypass,
                    replica_groups=[list(range(N_CORES))],
                    ins=[a2a_in[0].opt()], outs=[a2a_out[0].opt()])
                emit_attn(1, 2, att2, stp2, sc2, po2, colsum_mm=True)
                emit_attn(1, 3, att2, stp2, sc2, po2, colsum_mm=True)
                nc.gpsimd.collective_compute(
                    "AllToAll", mybir.AluOpType.bypass,
                    replica_groups=[list(range(N_CORES))],
                    ins=[a2a_in[1].opt()], outs=[a2a_out[1].opt()])

            # ================= scope 3: wo projection =================
            with tc.tile_pool(name="wsb", bufs=6) as wsb, \
                 tc.tile_pool(name="ofp", bufs=1) as ofp, \
                 tc.tile_pool(name="wps", bufs=1, space="PSUM") as wps:
                oTf = ofp.tile([128, NK, TPC], BF, tag="oTf")
                for half in range(2):
                    for sc in range(N_CORES):
                        nc.scalar.dma_start(
                            oTf[:, ds(sc * H_LOC + half * 2, 2), :],
                            a2a_out[half][sc].rearrange(
                                "(g p) t -> p g t", p=128))
                # et order: first the head-half delivered by AllToAll #0
                et_order = [sc * H_LOC + half * 2 + g
                            for half in range(2)
                            for sc in range(N_CORES) for g in range(2)]
                # 4 passes over d (1024 cols each); 8 psum banks = 4 tt x 2 d2
                for dp in range(4):
                    yps = [wps.tile([128, 512], F32, tag=f"yp{i}",
                                    name=f"yp{i}") for i in range(8)]
                    for ei, et in enumerate(et_order):
                        wot_t = wsb.tile([128, 1024], BF, tag="wot_t")
                        nc.sync.dma_start(
                            wot_t[:], wot.ap()[ts(et, 128), ts(dp, 1024)])
                        for tt in range(TPC // 128):
                            for d2 in range(2):
                                nc.tensor.matmul(
                                    yps[tt * 2 + d2][:],
                                    lhsT=oTf[:, et, ts(tt, 128)],
                                    rhs=wot_t[:, ts(d2, 512)],
                                    start=(ei == 0), stop=(ei == NK - 1))
                    for tt in range(TPC // 128):
                        for d2 in range(2):
                            yb = wsb.tile([128, 512], F32, tag="yb", bufs=2)
                            nc.vector.tensor_copy(yb[:], yps[tt * 2 + d2][:])
                            nc.scalar.dma_start(
                                out.ap()[ts(tt, 128),
                                         ds(dp * 1024 + d2 * 512, 512)],
                                yb[:])
    nc.compile()
    return nc


_NC_CACHE = None
_ROPE_PERM = np.concatenate(
    [np.arange(0, HD, 2), np.arange(1, HD, 2)])  # pair halves within a head


def _pack_inputs(x, wq, wk, wv, wo, gate, adapter, freqs_cos, freqs_sin,
                 mask):
    bf = ml_dtypes.bfloat16
    xf = np.ascontiguousarray(np.asarray(x, np.float32).reshape(TOK, D))
    mk = np.asarray(mask, np.float32).reshape(S, S)
    maskd = np.ascontiguousarray(np.stack(
        [mk[d * 128:(d + 1) * 128, d * 128:(d + 1) * 128].T
         for d in range(NCHB)])).astype(bf)
    wot = np.ascontiguousarray(np.asarray(wo, np.float32).T).astype(bf)
    adT = np.ascontiguousarray(
        np.asarray(adapter, np.float32).reshape(AL, D).T).astype(bf)
    gt = np.tanh(np.asarray(gate, np.float32).reshape(H))
    fc = np.ascontiguousarray(np.asarray(freqs_cos, np.float32)).astype(bf)
    fs = np.ascontiguousarray(np.asarray(freqs_sin, np.float32)).astype(bf)
    perm_full = (_ROPE_PERM[None, :]
                 + (np.arange(H_LOC) * HD)[:, None]).reshape(CH)
    in_maps = []
    for r in range(N_CORES):
        sl = slice(r * CH, (r + 1) * CH)
        wq_p = np.asarray(wq, np.float32)[sl][perm_full]
        wk_p = np.asarray(wk, np.float32)[sl][perm_full]
        wv_s = np.asarray(wv, np.float32)[sl]
        wt3 = np.ascontiguousarray(
            np.stack([wq_p.T, wk_p.T, wv_s.T])).astype(bf)
        in_maps.append({
            "x": xf,
            "wt3": wt3,
            "wot": wot,
            "maskd": maskd,
            "fc": fc,
            "fs": fs,
            "adT": adT,
            "gth": np.ascontiguousarray(
                gt[r * H_LOC:(r + 1) * H_LOC].reshape(1, H_LOC)),
        })
    return in_maps


def kernel(x, wq, wk, wv, wo, gate, adapter, freqs_cos, freqs_sin, mask,
           start_pos=0, **_unused):
    global _NC_CACHE
    if _NC_CACHE is None:
        _NC_CACHE = build()
    nc = _NC_CACHE
    in_maps = _pack_inputs(x, wq, wk, wv, wo, gate, adapter, freqs_cos,
                           freqs_sin, mask)
    res = bass_utils.run_bass_kernel_spmd(nc, in_maps,
                                          core_ids=list(range(N_CORES)))
    y = np.concatenate([res.results[r]["out"] for r in range(N_CORES)], axis=0)
    return y.reshape(B, S, D)


if __name__ == "__main__":
    nc = build()
    print("compiled ok, instrs:",
          sum(len(bb.instructions) for f in nc.m.functions for bb in f.blocks))


# revision 21
# speedup vs baseline: 1.3966x; 1.0404x over previous
"""Distributed Trainium2 kernel for the gated-adapter attention module.

Head-parallel tensor parallelism over 8 NeuronCores (4 heads each).
Weights are host-packed (transposed, bf16, RoPE-pair-permuted for q/k) so
the device only streams x in f32, computes QKV with 512-wide bf16
matmuls, applies RoPE on contiguous 64-lane halves, runs flash-style
causal attention per head with scores held transposed (keys on
partitions), and finishes with a split AllToAll (head-sharded ->
token-sharded, two head-halves so the collective overlaps the attention
tail) followed by the full wo projection per 512-token slice.  Batch-1
QKV is interleaved with batch-0 attention; softmax column sums use a
DVE tree in the tensor-bound region and ones-matmuls in the
attention-only tail; PSUM->SBUF copies ride the scalar engine.
"""

import sys

sys.path.insert(0, "/opt/trn_rl_repo")

import numpy as np
import ml_dtypes

import concourse.bass as bass
import concourse.mybir as mybir
import concourse.tile as tile
from concourse import bacc, bass_utils
from concourse.bass import ds, ts
from concourse.masks import make_identity

N_CORES = 8
B, S, D = 2, 2048, 4096
H = 32
HD = 128                      # head dim
H_LOC = H // N_CORES          # 4 heads per core
CH = H_LOC * HD               # 512 local channels
TOK = B * S                   # 4096 tokens
NK = D // 128                 # 32 contraction tiles
AL = 10                       # adapter length
TPC = TOK // N_CORES          # 512 tokens per core after AllToAll
NQC = S // 512                # 4 query chunks per sequence
NCHB = S // 128               # 16 token chunks per batch
SCALE = 1.0 / float(np.sqrt(HD))
BF = mybir.dt.bfloat16
F32 = mybir.dt.float32
EXP = mybir.ActivationFunctionType.Exp
COPY = mybir.ActivationFunctionType.Copy
MULT = mybir.AluOpType.mult
ADD = mybir.AluOpType.add


def build():
    nc = bacc.Bacc("TRN2", target_bir_lowering=False, debug=False,
                   num_devices=N_CORES)
    x = nc.dram_tensor("x", [TOK, D], F32, kind="ExternalInput")
    wt3 = nc.dram_tensor("wt3", [3, D, CH], BF, kind="ExternalInput")
    wot = nc.dram_tensor("wot", [D, D], BF, kind="ExternalInput")
    maskd = nc.dram_tensor("maskd", [NCHB, 128, 128], BF, kind="ExternalInput")
    fc = nc.dram_tensor("fc", [S, HD // 2], BF, kind="ExternalInput")
    fs = nc.dram_tensor("fs", [S, HD // 2], BF, kind="ExternalInput")
    adT = nc.dram_tensor("adT", [D, AL], BF, kind="ExternalInput")
    gth = nc.dram_tensor("gth", [1, H_LOC], F32, kind="ExternalInput")
    out = nc.dram_tensor("out", [TPC, D], F32, kind="ExternalOutput")

    with tile.TileContext(nc) as tc:
        with tc.tile_pool(name="dram", bufs=1, space="DRAM") as dram, \
             tc.tile_pool(name="persist", bufs=1) as persist:
            # q/k spilled transposed per (b, h): contiguous [128, S] reads
            qkT_d = dram.tile([2, B * H_LOC, HD, S], BF, tag="qkT_d")
            v_d = [dram.tile([S, CH], BF, tag=f"v{b}", name=f"v{b}")
                   for b in range(B)]
            # AllToAll split into two head-halves (h0/h1 vs h2/h3) so the
            # first collective overlaps the attention tail.
            a2a_in = [dram.tile([N_CORES, CH // 2, TPC], BF, tag=f"ai{i}",
                                name=f"ai{i}") for i in range(2)]
            a2a_out = [dram.tile([N_CORES, CH // 2, TPC], BF, tag=f"ao{i}",
                                 name=f"ao{i}") for i in range(2)]

            ident = persist.tile([128, 128], BF, tag="ident")
            make_identity(nc, ident[:])
            ones01 = persist.tile([128, 1], BF, tag="ones01")
            nc.vector.memset(ones01[:], 1.0)
            g_sb = persist.tile([128, H_LOC], F32, tag="g_sb")
            nc.scalar.dma_start(g_sb[:], gth.ap().partition_broadcast(128))
            cs_sb = persist.tile([128, NCHB, HD // 2], BF, tag="cs_sb")
            nc.scalar.dma_start(
                cs_sb[:], fc.ap().rearrange("(pb p) f -> p pb f", p=128))
            sn_sb = persist.tile([128, NCHB, HD // 2], BF, tag="sn_sb")
            nc.scalar.dma_start(
                sn_sb[:], fs.ap().rearrange("(pb p) f -> p pb f", p=128))
            maskT = persist.tile([128, NCHB, 128], BF, tag="maskT")
            nc.scalar.dma_start(
                maskT[:], maskd.ap().rearrange("d p q -> p d q"))
            aT = persist.tile([128, NK, AL], BF, tag="aT")
            nc.scalar.dma_start(
                aT[:], adT.ap().rearrange("(k p) a -> p k a", p=128))
            a_kT = persist.tile([128, H_LOC, AL], BF, tag="a_kT")
            a_v = persist.tile([AL, CH], BF, tag="a_v")

            def emit_attn(b_i, h, att, stp, sc_ps, po_ps, colsum_mm):
                """Flash attention for one (batch, local head)."""
                bh = b_i * H_LOC + h
                ldb = 4 if colsum_mm else 2
                qTb = att.tile([128, S], BF, tag="qTb", bufs=ldb)
                nc.sync.dma_start(qTb[:], qkT_d[0, bh])
                kTb = att.tile([128, S], BF, tag="kTb", bufs=ldb)
                nc.sync.dma_start(kTb[:], qkT_d[1, bh])
                vb2 = att.tile([128, NCHB, HD], BF, tag="vb2", bufs=ldb)
                nc.sync.dma_start(
                    vb2[:],
                    v_d[b_i][:, ts(h, HD)].rearrange(
                        "(kt p) d -> p kt d", p=128))
                for qc in range(NQC):
                    nkt = (qc + 1) * 4
                    o_ps = po_ps.tile([128, 512], F32, tag="o")
                    if colsum_mm:
                        s_ps = po_ps.tile([128, 512], F32, tag="s")
                    else:
                        acc = att.tile([128, 512], F32, tag="acc")
                        accb = att.tile([128, 512], BF, tag="accb")
                    for kt in range(nkt):
                        sps = sc_ps.tile([128, 512], F32, tag="sc")
                        nc.tensor.matmul(sps[:], lhsT=kTb[:, ts(kt, 128)],
                                         rhs=qTb[:, ts(qc, 512)],
                                         start=True, stop=True)
                        stb = stp.tile([128, 512], BF, tag="stb")
                        if kt // 4 == qc:
                            off = (kt % 4) * 128
                            if off > 0:
                                nc.vector.memset(stb[:, ds(0, off)], 0.0)
                            sd = stp.tile([128, 128], F32, tag="sd", bufs=2)
                            nc.vector.scalar_tensor_tensor(
                                sd[:], sps[:, ds(off, 128)], SCALE,
                                maskT[:, kt, :], op0=MULT, op1=ADD)
                            nc.scalar.activation(
                                stb[:, ds(off, 128)], sd[:], EXP)
                            if off + 128 < 512:
                                nc.scalar.activation(
                                    stb[:, ds(off + 128, 384 - off)],
                                    sps[:, ds(off + 128, 384 - off)],
                                    EXP, scale=SCALE)
                        else:
                            nc.scalar.activation(stb[:], sps[:], EXP,
                                                 scale=SCALE)
                        nc.tensor.matmul(o_ps[:], lhsT=vb2[:, kt, :],
                                         rhs=stb[:], start=(kt == 0),
                                         stop=(kt == nkt - 1))
                        if colsum_mm:
                            nc.tensor.matmul(s_ps[0:1, :],
                                             lhsT=ones01[:, 0:1], rhs=stb[:],
                                             start=(kt == 0),
                                             stop=(kt == nkt - 1))
                        elif kt == 0:
                            nc.vector.tensor_copy(acc[:], stb[:])
                        elif kt < nkt - 1:
                            nc.vector.tensor_add(acc[:], acc[:], stb[:])
                        else:
                            nc.vector.tensor_add(accb[:], acc[:], stb[:])
                    # adapter cross-attention (own softmax)
                    spa = sc_ps.tile([128, 512], F32, tag="sc")
                    nc.tensor.matmul(spa[:AL, :], lhsT=a_kT[:, h, :],
                                     rhs=qTb[:, ts(qc, 512)],
                                     start=True, stop=True)
                    pab = stp.tile([AL, 512], BF, tag="pab", bufs=2)
                    nc.scalar.activation(pab[:], spa[:AL, :], EXP,
                                         scale=SCALE)
                    oa_ps = po_ps.tile([128, 512], F32, tag="oa")
                    nc.tensor.matmul(oa_ps[:], lhsT=a_v[:, ts(h, HD)],
                                     rhs=pab[:], start=True, stop=True)
                    # denominators: ones-matmul column sums
                    if not colsum_mm:
                        s_ps = sc_ps.tile([128, 512], F32, tag="sc")
                        nc.tensor.matmul(s_ps[0:1, :], lhsT=ones01[:, 0:1],
                                         rhs=accb[:], start=True, stop=True)
                    sa2 = sc_ps.tile([128, 512], F32, tag="sc")
                    nc.tensor.matmul(sa2[0:1, :], lhsT=ones01[:AL, 0:1],
                                     rhs=pab[:], start=True, stop=True)
                    den = att.tile([1, 1024], F32, tag="den", bufs=1)
                    nc.vector.tensor_copy(den[:, 0:512], s_ps[0:1, :])
                    nc.vector.tensor_copy(den[:, 512:1024], sa2[0:1, :])
                    rden = att.tile([1, 1024], F32, tag="rden", bufs=1)
                    nc.vector.reciprocal_approx_fast(rden[:], den[:])
                    rbc = att.tile([128, 1024], F32, tag="rbc", bufs=2)
                    nc.gpsimd.partition_broadcast(rbc[:], rden[:])
                    t3 = att.tile([128, 512], F32, tag="t3", bufs=1)
                    nc.vector.tensor_mul(t3[:], o_ps[:], rbc[:, 0:512])
                    t4 = att.tile([128, 512], F32, tag="t4", bufs=1)
                    nc.vector.scalar_tensor_tensor(
                        t4[:], oa_ps[:], g_sb[:, ds(h, 1)],
                        rbc[:, 512:1024], op0=MULT, op1=MULT)
                    ob = att.tile([128, 512], BF, tag="ob")
                    nc.vector.tensor_add(ob[:], t3[:], t4[:])
                    nc.sync.dma_start(
                        a2a_in[h // 2][b_i * NQC + qc,
                                       ts(h % 2, HD), :], ob[:])

            # ======== scope 1: QKV (both batches) + attention b0 ========
            with tc.tile_pool(name="wtp", bufs=1) as wtp, \
                 tc.tile_pool(name="run", bufs=2) as run, \
                 tc.tile_pool(name="att", bufs=2) as att, \
                 tc.tile_pool(name="stp", bufs=4) as stp, \
                 tc.tile_pool(name="pp_ps", bufs=2, space="PSUM") as pp_ps, \
                 tc.tile_pool(name="tp_ps", bufs=2, space="PSUM") as tp_ps, \
                 tc.tile_pool(name="sc_ps", bufs=2, space="PSUM") as sc_ps, \
                 tc.tile_pool(name="po_ps", bufs=1, space="PSUM") as po_ps:
                wT = wtp.tile([128, 3, NK, CH], BF, tag="wT")
                for p_i in range(3):
                    nc.scalar.dma_start(
                        wT[:, p_i, :, :],
                        wt3.ap()[p_i].rearrange("(k p) c -> p k c", p=128))

                def emit_adapter():
                    # adapter projections: a_kT per head, a_v
                    for h in range(H_LOC):
                        pk = sc_ps.tile([128, 512], F32, tag="sc")
                        for dt in range(NK):
                            nc.tensor.matmul(pk[:, :AL],
                                             lhsT=wT[:, 1, dt, ts(h, HD)],
                                             rhs=aT[:, dt, :],
                                             start=(dt == 0),
                                             stop=(dt == NK - 1))
                        nc.vector.tensor_copy(a_kT[:, h, :], pk[:, :AL])
                    pv = sc_ps.tile([128, 512], F32, tag="sc")
                    for dt in range(NK):
                        nc.tensor.matmul(pv[:AL, :], lhsT=aT[:, dt, :],
                                         rhs=wT[:, 2, dt, :], start=(dt == 0),
                                         stop=(dt == NK - 1))
                    nc.vector.tensor_copy(a_v[:], pv[:AL, :])

                def emit_chunk(b_i, c16):
                    """QKV + RoPE + spills for one 128-token chunk."""
                    tstr = b_i * NCHB + c16
                    xT = run.tile([128, NK, 128], BF, tag="xT")
                    for hf in range(4):
                        xf = run.tile([128, D // 4], F32, tag="xf")
                        nc.sync.dma_start(
                            xf[:], x.ap()[ts(tstr, 128), ts(hf, D // 4)])
                        xb = run.tile([128, D // 4], BF, tag="xb")
                        nc.vector.tensor_copy(xb[:], xf[:])
                        tps = tp_ps.tile([128, 1024], BF, tag="tp")
                        for j in range(8):
                            nc.tensor.transpose(
                                tps[:, ts(j, 128)], xb[:, ts(j, 128)],
                                ident[:])
                        nc.vector.tensor_copy(
                            xT[:, ds(hf * 8, 8), :].rearrange(
                                "p a b -> p (a b)"), tps[:])
                    csb = cs_sb[:, c16, None, :].broadcast_to([128, H_LOC, 64])
                    snb = sn_sb[:, c16, None, :].broadcast_to([128, H_LOC, 64])
                    for p_i in range(3):
                        pp = pp_ps.tile([128, CH], F32, tag="pp")
                        for dt in range(NK):
                            nc.tensor.matmul(pp[:], lhsT=xT[:, dt, :],
                                             rhs=wT[:, p_i, dt, :],
                                             start=(dt == 0),
                                             stop=(dt == NK - 1))
                        if p_i == 2:
                            vb = run.tile([128, CH], BF, tag="vb")
                            nc.vector.tensor_copy(vb[:], pp[:])
                            nc.sync.dma_start(
                                v_d[b_i][ts(c16, 128), :], vb[:])
                            return
                        ppv = pp[:].rearrange("p (h i) -> p h i", h=H_LOC)
                        pa, pb = ppv[:, :, 0:64], ppv[:, :, 64:128]
                        t1 = run.tile([128, H_LOC, 64], F32, tag="t1")
                        t2 = run.tile([128, H_LOC, 64], F32, tag="t2")
                        rq = run.tile([128, CH], BF, tag=f"rq{p_i}",
                                      name=f"rq{p_i}")
                        rqv = rq[:].rearrange("p (h i) -> p h i", h=H_LOC)
                        nc.vector.tensor_mul(t1[:], pa, csb)
                        nc.vector.tensor_mul(t2[:], pb, snb)
                        nc.vector.tensor_sub(rqv[:, :, 0:64], t1[:], t2[:])
                        nc.vector.tensor_mul(t1[:], pa, snb)
                        nc.vector.tensor_mul(t2[:], pb, csb)
                        nc.vector.tensor_add(rqv[:, :, 64:128], t1[:], t2[:])
                        tps = tp_ps.tile([128, 1024], BF, tag="tp")
                        for h in range(H_LOC):
                            nc.tensor.transpose(
                                tps[:, ts(h, 128)], rq[:, ts(h, HD)], ident[:])
                        stg = run.tile([128, 512], BF, tag=f"st{p_i}",
                                       name=f"st{p_i}")
                        nc.vector.tensor_copy(stg[:], tps[:, 0:512])
                        nc.sync.dma_start(
                            qkT_d[p_i, ds(b_i * H_LOC, H_LOC), :,
                                  ts(c16, 128)].rearrange("h p t -> p h t"),
                            stg[:].rearrange("p (h t) -> p h t", h=H_LOC))

                # batch 0 QKV, then batch 1 QKV interleaved with batch 0
                # attention
                for c16 in range(NCHB):
                    emit_chunk(0, c16)
                    if c16 == 1:
                        emit_adapter()
                for grp in range(4):
                    for c16 in range(grp * 4, grp * 4 + 4):
                        emit_chunk(1, c16)
                    emit_attn(0, grp, att, stp, sc_ps, po_ps,
                              colsum_mm=False)

            # ======== scope 2: attention b1 tail + split AllToAll ========
            with tc.tile_pool(name="att2", bufs=2) as att2, \
                 tc.tile_pool(name="stp2", bufs=4) as stp2, \
                 tc.tile_pool(name="sc2_ps", bufs=4, space="PSUM") as sc2, \
                 tc.tile_pool(name="po2_ps", bufs=1, space="PSUM") as po2:
                emit_attn(1, 0, att2, stp2, sc2, po2, colsum_mm=True)
                emit_attn(1, 1, att2, stp2, sc2, po2, colsum_mm=True)
                nc.gpsimd.collective_compute(
                    "AllToAll", mybir.AluOpType.bypass,
                    replica_groups=[list(range(N_CORES))],
                    ins=[a2a_in[0].opt()], outs=[a2a_out[0].opt()])
                emit_attn(1, 2, att2, stp2, sc2, po2, colsum_mm=True)
                emit_attn(1, 3, att2, stp2, sc2, po2, colsum_mm=True)
                nc.gpsimd.collective_compute(
                    "AllToAll", mybir.AluOpType.bypass,
                    replica_groups=[list(range(N_CORES))],
                    ins=[a2a_in[1].opt()], outs=[a2a_out[1].opt()])

            # ================= scope 3: wo projection =================
            with tc.tile_pool(name="wsb", bufs=6) as wsb, \
                 tc.tile_pool(name="ofp", bufs=1) as ofp, \
                 tc.tile_pool(name="wps", bufs=1, space="PSUM") as wps:
                oTf = ofp.tile([128, NK, TPC], BF, tag="oTf")
                for half in range(2):
                    for sc in range(N_CORES):
                        nc.scalar.dma_start(
                            oTf[:, ds(sc * H_LOC + half * 2, 2), :],
                            a2a_out[half][sc].rearrange(
                                "(g p) t -> p g t", p=128))
                # et order: first the head-half delivered by AllToAll #0
                et_order = [sc * H_LOC + half * 2 + g
                            for half in range(2)
                            for sc in range(N_CORES) for g in range(2)]
                # 4 passes over d (1024 cols each); 8 psum banks = 4 tt x 2 d2
                for dp in range(4):
                    yps = [wps.tile([128, 512], F32, tag=f"yp{i}",
                                    name=f"yp{i}") for i in range(8)]
                    for ei, et in enumerate(et_order):
                        wot_t = wsb.tile([128, 1024], BF, tag="wot_t")
                        nc.sync.dma_start(
                            wot_t[:], wot.ap()[ts(et, 128), ts(dp, 1024)])
                        for tt in range(TPC // 128):
                            for d2 in range(2):
                                nc.tensor.matmul(
                                    yps[tt * 2 + d2][:],
                                    lhsT=oTf[:, et, ts(tt, 128)],
                                    rhs=wot_t[:, ts(d2, 512)],
                                    start=(ei == 0), stop=(ei == NK - 1))
                    for tt in range(TPC // 128):
                        for d2 in range(2):
                            yb = wsb.tile([128, 512], F32, tag="yb", bufs=2)
                            nc.vector.tensor_copy(yb[:], yps[tt * 2 + d2][:])
                            nc.scalar.dma_start(
                                out.ap()[ts(tt, 128),
                                         ds(dp * 1024 + d2 * 512, 512)],
                                yb[:])
    nc.compile()
    return nc


_NC_CACHE = None
_ROPE_PERM = np.concatenate(
    [np.arange(0, HD, 2), np.arange(1, HD, 2)])  # pair halves within a head


def _pack_inputs(x, wq, wk, wv, wo, gate, adapter, freqs_cos, freqs_sin,
                 mask):
    bf = ml_dtypes.bfloat16
    xf = np.ascontiguousarray(np.asarray(x, np.float32).reshape(TOK, D))
    mk = np.asarray(mask, np.float32).reshape(S, S)
    maskd = np.ascontiguousarray(np.stack(
        [mk[d * 128:(d + 1) * 128, d * 128:(d + 1) * 128].T
         for d in range(NCHB)])).astype(bf)
    wot = np.ascontiguousarray(np.asarray(wo, np.float32).T).astype(bf)
    adT = np.ascontiguousarray(
        np.asarray(adapter, np.float32).reshape(AL, D).T).astype(bf)
    gt = np.tanh(np.asarray(gate, np.float32).reshape(H))
    fc = np.ascontiguousarray(np.asarray(freqs_cos, np.float32)).astype(bf)
    fs = np.ascontiguousarray(np.asarray(freqs_sin, np.float32)).astype(bf)
    perm_full = (_ROPE_PERM[None, :]
                 + (np.arange(H_LOC) * HD)[:, None]).reshape(CH)
    in_maps = []
    for r in range(N_CORES):
        sl = slice(r * CH, (r + 1) * CH)
        wq_p = np.asarray(wq, np.float32)[sl][perm_full]
        wk_p = np.asarray(wk, np.float32)[sl][perm_full]
        wv_s = np.asarray(wv, np.float32)[sl]
        wt3 = np.ascontiguousarray(
            np.stack([wq_p.T, wk_p.T, wv_s.T])).astype(bf)
        in_maps.append({
            "x": xf,
            "wt3": wt3,
            "wot": wot,
            "maskd": maskd,
            "fc": fc,
            "fs": fs,
            "adT": adT,
            "gth": np.ascontiguousarray(
                gt[r * H_LOC:(r + 1) * H_LOC].reshape(1, H_LOC)),
        })
    return in_maps


def kernel(x, wq, wk, wv, wo, gate, adapter, freqs_cos, freqs_sin, mask,
           start_pos=0, **_unused):
    global _NC_CACHE
    if _NC_CACHE is None:
        _NC_CACHE = build()
    nc = _NC_CACHE
    in_maps = _pack_inputs(x, wq, wk, wv, wo, gate, adapter, freqs_cos,
                           freqs_sin, mask)
    res = bass_utils.run_bass_kernel_spmd(nc, in_maps,
                                          core_ids=list(range(N_CORES)))
    y = np.concatenate([res.results[r]["out"] for r in range(N_CORES)], axis=0)
    return y.reshape(B, S, D)


if __name__ == "__main__":
    nc = build()
    print("compiled ok, instrs:",
          sum(len(bb.instructions) for f in nc.m.functions for bb in f.blocks))
